# revision 3
# baseline (speedup 1.0000x reference)
"""Trainium2 kernel for nn_BatchedDTW — bf16 stream + PE group-reduce.

Math (from the reference's recurrence): both DTW predecessors live on row
i-1, so every path step is diagonal and
    out = mean_{b,n} sum_t ||X[b,t,n,:] - Y[b,t,n,:]||_2
— a pure streaming reduction over the (B*T*N, C=32) rows.  The harness
tolerance is rel_err < 2e-2; quantizing the inputs to bf16 perturbs each
distance by ~0.2% of random jitter that averages out across the 131072 rows
(measured end-to-end error ~1e-4), so the kernel streams bf16 and halves the
HBM traffic that bounds this memory-regime problem: 2 MiB per core per pass.

Layout (per core): rows R = 0..16383, channels c = 0..31.  Partition
p = 32*(R%4) + c, free column q = R//4 — i.e. channels live on the partition
axis (4 rows per 128-partition column).  Elementwise stages don't care, and
it lets the PE array do the per-row 32-channel reduction as a matmul with a
constant block-ones stationary:
    psum[32*b + 4*i + r, n] = sum_c sq[32*r + c, 256*(4*b+i) + n]
via 16 matmuls (256 moving cols each) of W_i[k, m] = 1{m == 4*i + k//32},
i = s%4, accumulated 4-to-a-quadrant (start at i==0) into psum[128, 256].

Engines per rep (measured HW DMA rate ~360-375 GB/s/core; a DMA-only probe
of the same 2 MiB stream runs ~5.6 us/rep, and the full kernel measures the
same within noise — the pipeline is DMA-bound with compute hidden):
  SP   issues 2 x 1 MiB chunk DMAs (HWDGE qSP)           ~5.6 us (bound)
  DVE  sub (bf16 2x) + first SD cols of squares          ~3.0 us
  ACT  remaining squares + sqrt(psum)+accum (one set)    ~2.9 us
  PE   16 block-ones matmuls -> psum                     ~2.1 us
The f32 predecessor (4 MiB/rep, DVE sub + ACT squares + DVE grouped reduce)
measures ~12.1 us/rep under the same protocol — bf16 + PE-reduce is ~2.4x.
"""

from contextlib import ExitStack

import numpy as np
import ml_dtypes

import concourse.bass as bass
import concourse.mybir as mybir
from concourse.bass_utils import run_bass_kernel_spmd

N_CORES = 8
P = 128
C = 32
B, T, N = 4, 512, 64
ROWS = B * T * N // N_CORES   # 16384 rows per core
F = ROWS // 4                 # 4096 free cols per partition (4 rows/column)
NCH = 2                       # input chunks per rep
FC = F // NCH                 # cols per chunk
SL = 256                      # moving cols per matmul
NSL = F // SL                 # 16 matmuls per rep
SD = 1280                     # cols squared on DVE (rest on ACT); 5 slices
KSETS = 4                     # DMA sem ring depth

assert SD % SL == 0

_nc_cache = {}
_last_results = None


def _build(repeat=1, sd=SD, nch=NCH, nbuf=None, stages="smqpr", sl=SL):
    """stages: which pipeline stages to emit (ablation diagnostics).
    's' sub, 'm' DVE mul, 'q' ACT square, 'p' PE matmuls, 'r' ACT sqrt.
    Later stages require earlier ones; with stages missing, downstream waits
    are dropped and the close-out gates on the last present stage."""
    if nbuf is None:
        nbuf = 2 if repeat > 1 else 1
    S, M, Q, PE_, R_ = (c in stages for c in "smqpr")
    nsl = F // sl                 # matmuls per rep
    half = nsl // 2               # accumulating matmuls per psum half
    assert sd % sl == 0
    fc = F // nch
    nc = bass.Bass()
    bf16 = mybir.dt.bfloat16
    f32 = mybir.dt.float32
    z_ext = nc.declare_dram_parameter("z", [P, 2 * F], bf16, isOutput=False)
    w_ext = nc.declare_dram_parameter("w", [P, 8 * 64], bf16, isOutput=False)
    out_ext = nc.declare_dram_parameter("out", [P, 1], f32, isOutput=True)

    ksets = min(repeat, KSETS)
    with ExitStack() as ctx:
        zt = ctx.enter_context(nc.sbuf_tensor([P, nbuf * 2 * F], bf16))
        df = ctx.enter_context(nc.sbuf_tensor([P, nbuf * F], bf16))
        sq = ctx.enter_context(nc.sbuf_tensor([P, nbuf * F], bf16))
        wt = ctx.enter_context(nc.sbuf_tensor([P, 8 * 64], bf16))
        acc = ctx.enter_context(nc.sbuf_tensor([P, nbuf], f32))
        ps = ctx.enter_context(nc.psum_tensor([P, nbuf * 2 * sl], f32))
        zsems = [ctx.enter_context(nc.semaphore(f"zsem{r}_{i}"))
                 for r in range(ksets) for i in range(nch)]
        wsem = ctx.enter_context(nc.semaphore("wsem"))
        vsem = ctx.enter_context(nc.semaphore("vsem"))
        asem = ctx.enter_context(nc.semaphore("asem"))
        psem = ctx.enter_context(nc.semaphore("psem"))
        osem = ctx.enter_context(nc.semaphore("osem"))
        block = ctx.enter_context(nc.Block())

        def zs(r, ch):
            return zsems[(r % ksets) * nch + ch]

        def z_done(r):
            return 16 * (r // ksets + 1)

        def zoff(r):
            return (r % nbuf) * 2 * F

        def foff(r):
            return (r % nbuf) * F

        def poff(r):
            return (r % nbuf) * 2 * sl

        nv = int(S) + int(M)    # vsem incs per rep (sub, mul)
        na = int(Q) + int(R_)   # asem incs per rep (square, sqrt)

        def v_sub_done(r):
            return nv * r + 1

        def v_mul_done(r):
            return nv * r + nv

        def a_sq_done(r):
            return na * r + 1

        def a_sqrt_done(r):
            return na * r + na

        # psem: one inc per matmul
        def p_done(r, s):
            return nsl * r + s + 1

        @block.sync
        def _(sync):
            sync.dma_start(out=wt[:], in_=w_ext[:]).then_inc(wsem, 16)
            for r in range(repeat):
                for ch in range(nch):
                    if S and r >= nbuf:
                        # WAR: rep r reuses rep r-nbuf's zt slot; its sub
                        # (which reads the whole slot) must have finished
                        sync.wait_ge(vsem, v_sub_done(r - nbuf))
                    sync.dma_start(
                        out=zt[:, zoff(r) + ch * 2 * fc:
                               zoff(r) + (ch + 1) * 2 * fc],
                        in_=z_ext[:, ch * 2 * fc:(ch + 1) * 2 * fc],
                    ).then_inc(zs(r, ch), 16)

        @block.vector
        def _(vector):
            for r in range(repeat):
                if S:
                    for ch in range(nch):
                        vector.wait_ge(zs(r, ch), z_done(r))
                    if Q and r >= nbuf:
                        # WAR: df slot last read by ACT's square of rep r-nbuf
                        vector.wait_ge(asem, a_sq_done(r - nbuf))
                    # df = x - y over the rep (chunks are x|y interleaved)
                    vector.tensor_sub(
                        df[:, foff(r):foff(r) + F]
                        .rearrange("p (ch f) -> p ch f", ch=nch),
                        zt[:, zoff(r):zoff(r) + 2 * F]
                        .rearrange("p (ch xy f) -> p ch xy f", ch=nch, xy=2)
                        [:, :, 0],
                        zt[:, zoff(r):zoff(r) + 2 * F]
                        .rearrange("p (ch xy f) -> p ch xy f", ch=nch, xy=2)
                        [:, :, 1],
                    ).then_inc(vsem, 1)
                if M:
                    if PE_ and r >= nbuf:
                        # WAR: sq cols [0, sd) last read by PE of rep r-nbuf
                        vector.wait_ge(psem, p_done(r - nbuf, sd // sl - 1))
                    vector.tensor_mul(
                        sq[:, foff(r):foff(r) + sd],
                        df[:, foff(r):foff(r) + sd],
                        df[:, foff(r):foff(r) + sd],
                    ).then_inc(vsem, 1)

        @block.scalar
        def _(scalar):
            for r in range(repeat):
                if Q:
                    scalar.wait_ge(vsem, v_sub_done(r))
                    if PE_ and r >= nbuf:
                        # WAR: sq cols [sd, F) last read by PE of rep r-nbuf
                        scalar.wait_ge(psem, p_done(r - nbuf, nsl - 1))
                    scalar.square(
                        out=sq[:, foff(r) + sd:foff(r) + F],
                        in_=df[:, foff(r) + sd:foff(r) + F],
                    ).then_inc(asem, 1)
                if R_:
                    scalar.wait_ge(psem, p_done(r, nsl - 1))
                    scalar.activation(
                        out=ps[:, poff(r) + sl:poff(r) + 2 * sl],
                        in_=ps[:, poff(r):poff(r) + sl],
                        func=mybir.ActivationFunctionType.Sqrt,
                        accum_out=acc[:, r % nbuf:r % nbuf + 1],
                    ).then_inc(asem, 1)
            # close-out: gate on the last present stage, then emit the output
            if R_:
                scalar.wait_ge(asem, a_sqrt_done(repeat - 1))
            elif PE_:
                scalar.wait_ge(psem, p_done(repeat - 1, nsl - 1))
            elif Q:
                scalar.wait_ge(asem, na * repeat)
            elif M or S:
                scalar.wait_ge(vsem, nv * repeat)
            else:
                for st in range(ksets):
                    if repeat > st:
                        for ch in range(nch):
                            scalar.wait_ge(zsems[st * nch + ch],
                                           16 * ((repeat - 1 - st) // ksets + 1))
            scalar.dma_start(
                out=out_ext[:],
                in_=acc[:, (repeat - 1) % nbuf:(repeat - 1) % nbuf + 1],
            ).then_inc(osem, 16)
            scalar.wait_ge(osem, 16)

        if PE_:
            @block.tensor
            def _(tensor):
                tensor.wait_ge(wsem, 16)
                for r in range(repeat):
                    for s in range(nsl):
                        h, i = s // half, s % half
                        if M and s * sl == 0:
                            # producer: DVE's tensor_mul covers cols [0, sd)
                            tensor.wait_ge(vsem, v_mul_done(r))
                        elif Q and s * sl == sd:
                            # producer: ACT's square covers cols [sd, F)
                            tensor.wait_ge(asem, a_sq_done(r))
                        if R_ and i == 0 and r >= nbuf:
                            # WAR: psum half reset; sqrt of rep r-nbuf must
                            # have read it
                            tensor.wait_ge(asem, a_sqrt_done(r - nbuf))
                        tensor.matmul(
                            out=ps[64 * h:64 * (h + 1), poff(r):poff(r) + sl],
                            lhsT=wt[:, 64 * i:64 * (i + 1)],
                            rhs=sq[:, foff(r) + s * sl:foff(r) + (s + 1) * sl],
                            start=(i == 0),
                            stop=(i == half - 1),
                        ).then_inc(psem, 1)
    return nc


def make_weights():
    # W_i[k, m] = 1{m == 4*i + k//32}, packed at free cols [64*i, 64*i+64)
    w = np.zeros((P, 8 * 64), dtype=np.float32)
    k = np.arange(P)
    for i in range(8):
        w[k, 64 * i + 4 * i + k // C] = 1.0
    return w.astype(ml_dtypes.bfloat16)


def pack_inputs(X, Y, nch=NCH):
    """(B,T,N,C) x2 -> per-core bf16 z arrays, channels-on-partition layout,
    x|y interleaved per chunk."""
    fc = F // nch

    def to_parts(A):
        A = np.asarray(A, dtype=np.float32).reshape(N_CORES, F, 4, C)
        # partition p = 32*(R%4) + c ; free col q = R//4
        return A.transpose(0, 2, 3, 1).reshape(N_CORES, P, F)

    Xp, Yp = to_parts(X), to_parts(Y)
    Z = np.empty((N_CORES, P, 2 * F), dtype=np.float32)
    for ch in range(nch):
        Z[:, :, 2 * ch * fc:(2 * ch + 1) * fc] = Xp[:, :, ch * fc:(ch + 1) * fc]
        Z[:, :, (2 * ch + 1) * fc:(2 * ch + 2) * fc] = Yp[:, :, ch * fc:(ch + 1) * fc]
    return Z.astype(ml_dtypes.bfloat16)


def kernel(X, Y, window=None, **_):
    global _nc_cache, _last_results
    Z = pack_inputs(X, Y)
    W = make_weights()
    if "k" not in _nc_cache:
        _nc_cache["k"] = _build()
    in_maps = [{"z": Z[k], "w": W} for k in range(N_CORES)]
    res = run_bass_kernel_spmd(_nc_cache["k"], in_maps, list(range(N_CORES)))
    _last_results = res
    partials = np.stack([r["out"] for r in res.results])  # (8, 128, 1)
    total = partials.astype(np.float64).sum()
    return np.float32(total / (B * N))


# revision 4
# speedup vs baseline: 1.5422x; 1.5422x over previous
"""Trainium2 kernel for nn_BatchedDTW — fp8 HBM stream (HWDGE), DVE sub\nat 1x fp8->bf16, ACT squares+sqrt, PE group-reduce.

Math (from the reference's recurrence): both DTW predecessors live on row
i-1, so every path step is diagonal and
    out = mean_{b,n} sum_t ||X[b,t,n,:] - Y[b,t,n,:]||_2
— a pure streaming reduction over the (B*T*N, C=32) rows.  The harness
tolerance is rel_err < 2e-2; quantizing the inputs to bf16 perturbs each
distance by ~0.2% of random jitter that averages out across the 131072 rows
(measured end-to-end error ~1e-4), so the kernel streams bf16 and halves the
HBM traffic that bounds this memory-regime problem: 2 MiB per core per pass.

Layout (per core): rows R = 0..16383, channels c = 0..31.  Partition
p = 32*(R%4) + c, free column q = R//4 — i.e. channels live on the partition
axis (4 rows per 128-partition column).  Elementwise stages don't care, and
it lets the PE array do the per-row 32-channel reduction as a matmul with a
constant block-ones stationary:
    psum[32*b + 4*i + r, n] = sum_c sq[32*r + c, 256*(4*b+i) + n]
via 16 matmuls (256 moving cols each) of W_i[k, m] = 1{m == 4*i + k//32},
i = s%4, accumulated 4-to-a-quadrant (start at i==0) into psum[128, 256].

Engines per rep (steady-state busy at the default split):
  SP   issues 2 x 1 MiB chunk DMAs (HWDGE qSP)           ~2.8 us (bound)
  DVE  sub (bf16 2x) + first SD cols of squares          ~2.9 us
  ACT  remaining squares + sqrt(psum)+accum (one set)    ~2.9 us
  PE   16 block-ones matmuls -> psum                     ~1.9 us
"""

from contextlib import ExitStack

import numpy as np
import ml_dtypes

import concourse.bass as bass
import concourse.mybir as mybir
from concourse.bass_utils import run_bass_kernel_spmd

N_CORES = 8
P = 128
C = 32
B, T, N = 4, 512, 64
ROWS = B * T * N // N_CORES   # 16384 rows per core
F = ROWS // 4                 # 4096 free cols per partition (4 rows/column)
NCH = 2                       # input chunks per rep
FC = F // NCH                 # cols per chunk
SL = 256                      # moving cols per matmul
NSL = F // SL                 # 16 matmuls per rep
SD = 0                        # all squares on ACT; fp8 sub saturates DVE
KSETS = 4                     # DMA sem ring depth

assert SD % SL == 0

_nc_cache = {}
_last_results = None


def _build(repeat=1, sd=SD, nch=NCH, nbuf=None, stages="sqpr", sl=SL):
    """stages: which pipeline stages to emit (ablation diagnostics).
    's' sub, 'm' DVE mul, 'q' ACT square, 'p' PE matmuls, 'r' ACT sqrt.
    Later stages require earlier ones; with stages missing, downstream waits
    are dropped and the close-out gates on the last present stage."""
    if nbuf is None:
        nbuf = 2 if repeat > 1 else 1
    S, M, Q, PE_, R_ = (c in stages for c in "smqpr")
    nsl = F // sl                 # matmuls per rep
    half = nsl // 2               # accumulating matmuls per psum half
    assert sd % sl == 0
    fc = F // nch
    nc = bass.Bass()
    bf16 = mybir.dt.bfloat16
    f32 = mybir.dt.float32
    f8 = mybir.dt.float8e4
    z_ext = nc.declare_dram_parameter("z", [P, 2 * F], f8, isOutput=False)
    w_ext = nc.declare_dram_parameter("w", [P, 8 * 64], bf16, isOutput=False)
    out_ext = nc.declare_dram_parameter("out", [P, 1], f32, isOutput=True)

    ksets = min(repeat, KSETS)
    with ExitStack() as ctx:
        zt = ctx.enter_context(nc.sbuf_tensor([P, nbuf * 2 * F], f8))
        df = ctx.enter_context(nc.sbuf_tensor([P, nbuf * F], bf16))
        sq = ctx.enter_context(nc.sbuf_tensor([P, nbuf * F], bf16))
        wt = ctx.enter_context(nc.sbuf_tensor([P, 8 * 64], bf16))
        acc = ctx.enter_context(nc.sbuf_tensor([P, nbuf], f32))
        ps = ctx.enter_context(nc.psum_tensor([P, nbuf * 2 * sl], f32))
        zsems = [ctx.enter_context(nc.semaphore(f"zsem{r}_{i}"))
                 for r in range(ksets) for i in range(nch)]
        wsem = ctx.enter_context(nc.semaphore("wsem"))
        vsem = ctx.enter_context(nc.semaphore("vsem"))
        asem = ctx.enter_context(nc.semaphore("asem"))
        psem = ctx.enter_context(nc.semaphore("psem"))
        osem = ctx.enter_context(nc.semaphore("osem"))
        block = ctx.enter_context(nc.Block())

        def zs(r, ch):
            return zsems[(r % ksets) * nch + ch]

        def z_done(r):
            return 16 * (r // ksets + 1)

        def zoff(r):
            return (r % nbuf) * 2 * F

        def foff(r):
            return (r % nbuf) * F

        def poff(r):
            return (r % nbuf) * 2 * sl

        nv = int(S) + int(M)    # vsem incs per rep (sub, mul)
        na = int(Q) + int(R_)   # asem incs per rep (square, sqrt)
        pipel = Q and R_        # ACT software-pipelined: sq(r) then sqrt(r-1)

        def v_sub_done(r):
            return nv * r + 1

        def v_mul_done(r):
            return nv * r + nv

        def a_sq_done(r):
            if pipel:
                # ACT order: sq0, [sq1, sqrt0], [sq2, sqrt1], ..., sqrt(R-1)
                return 1 if r == 0 else 2 * r

            return na * r + 1

        def a_sqrt_done(r):
            if pipel:
                return 2 * repeat if r == repeat - 1 else 2 * r + 3
            return na * r + na

        # psem: one inc per matmul
        def p_done(r, s):
            return nsl * r + s + 1

        @block.sync
        def _(sync):
            sync.dma_start(out=wt[:], in_=w_ext[:]).then_inc(wsem, 16)
            for r in range(repeat):
                for ch in range(nch):
                    if S and r >= nbuf:
                        # WAR: rep r reuses rep r-nbuf's zt slot; its sub
                        # (which reads the whole slot) must have finished
                        sync.wait_ge(vsem, v_sub_done(r - nbuf))
                    sync.dma_start(
                        out=zt[:, zoff(r) + ch * 2 * fc:
                               zoff(r) + (ch + 1) * 2 * fc],
                        in_=z_ext[:, ch * 2 * fc:(ch + 1) * 2 * fc],
                    ).then_inc(zs(r, ch), 16)

        @block.vector
        def _(vector):
            for r in range(repeat):
                if S:
                    for ch in range(nch):
                        vector.wait_ge(zs(r, ch), z_done(r))
                    if Q and r >= nbuf:
                        # WAR: df slot last read by ACT's square of rep r-nbuf
                        vector.wait_ge(asem, a_sq_done(r - nbuf))
                    # df = x - y over the rep (chunks are x|y interleaved)
                    vector.tensor_sub(
                        df[:, foff(r):foff(r) + F]
                        .rearrange("p (ch f) -> p ch f", ch=nch),
                        zt[:, zoff(r):zoff(r) + 2 * F]
                        .rearrange("p (ch xy f) -> p ch xy f", ch=nch, xy=2)
                        [:, :, 0],
                        zt[:, zoff(r):zoff(r) + 2 * F]
                        .rearrange("p (ch xy f) -> p ch xy f", ch=nch, xy=2)
                        [:, :, 1],
                    ).then_inc(vsem, 1)
                if M:
                    if PE_ and r >= nbuf:
                        # WAR: sq cols [0, sd) last read by PE of rep r-nbuf
                        vector.wait_ge(psem, p_done(r - nbuf, sd // sl - 1))
                    vector.tensor_mul(
                        sq[:, foff(r):foff(r) + sd],
                        df[:, foff(r):foff(r) + sd],
                        df[:, foff(r):foff(r) + sd],
                    ).then_inc(vsem, 1)

        @block.scalar
        def _(scalar):
            def emit_square(r):
                scalar.wait_ge(vsem, v_sub_done(r))
                if PE_ and r >= nbuf:
                    # WAR: sq cols [sd, F) last read by PE of rep r-nbuf
                    scalar.wait_ge(psem, p_done(r - nbuf, nsl - 1))
                scalar.square(
                    out=sq[:, foff(r) + sd:foff(r) + F],
                    in_=df[:, foff(r) + sd:foff(r) + F],
                ).then_inc(asem, 1)

            def emit_sqrt(r):
                scalar.wait_ge(psem, p_done(r, nsl - 1))
                scalar.activation(
                    out=ps[:, poff(r) + sl:poff(r) + 2 * sl],
                    in_=ps[:, poff(r):poff(r) + sl],
                    func=mybir.ActivationFunctionType.Sqrt,
                    accum_out=acc[:, r % nbuf:r % nbuf + 1],
                ).then_inc(asem, 1)

            if pipel:
                # sqrt lags its rep by one so it never blocks the next
                # square behind the PE matmuls it waits on
                for r in range(repeat):
                    emit_square(r)
                    if r >= 1:
                        emit_sqrt(r - 1)
                emit_sqrt(repeat - 1)
            else:
                for r in range(repeat):
                    if Q:
                        emit_square(r)
                    if R_:
                        emit_sqrt(r)
            # close-out: gate on the last present stage, then emit the output
            if R_:
                scalar.wait_ge(asem, a_sqrt_done(repeat - 1))
            elif PE_:
                scalar.wait_ge(psem, p_done(repeat - 1, nsl - 1))
            elif Q:
                scalar.wait_ge(asem, na * repeat)
            elif M or S:
                scalar.wait_ge(vsem, nv * repeat)
            else:
                for st in range(ksets):
                    if repeat > st:
                        for ch in range(nch):
                            scalar.wait_ge(zsems[st * nch + ch],
                                           16 * ((repeat - 1 - st) // ksets + 1))
            scalar.dma_start(
                out=out_ext[:],
                in_=acc[:, (repeat - 1) % nbuf:(repeat - 1) % nbuf + 1],
            ).then_inc(osem, 16)
            scalar.wait_ge(osem, 16)

        if PE_:
            @block.tensor
            def _(tensor):
                tensor.wait_ge(wsem, 16)
                for r in range(repeat):
                    for s in range(nsl):
                        h, i = s // half, s % half
                        if M and s * sl == 0:
                            # producer: DVE's tensor_mul covers cols [0, sd)
                            tensor.wait_ge(vsem, v_mul_done(r))
                        elif Q and s * sl == sd:
                            # producer: ACT's square covers cols [sd, F)
                            tensor.wait_ge(asem, a_sq_done(r))
                        if R_ and i == 0 and r >= nbuf:
                            # WAR: psum half reset; sqrt of rep r-nbuf must
                            # have read it
                            tensor.wait_ge(asem, a_sqrt_done(r - nbuf))
                        tensor.matmul(
                            out=ps[64 * h:64 * (h + 1), poff(r):poff(r) + sl],
                            lhsT=wt[:, 64 * i:64 * (i + 1)],
                            rhs=sq[:, foff(r) + s * sl:foff(r) + (s + 1) * sl],
                            start=(i == 0),
                            stop=(i == half - 1),
                        ).then_inc(psem, 1)
    return nc


def make_weights():
    # W_i[k, m] = 1{m == 4*i + k//32}, packed at free cols [64*i, 64*i+64)
    w = np.zeros((P, 8 * 64), dtype=np.float32)
    k = np.arange(P)
    for i in range(8):
        w[k, 64 * i + 4 * i + k // C] = 1.0
    return w.astype(ml_dtypes.bfloat16)


def pack_inputs(X, Y, nch=NCH):
    """(B,T,N,C) x2 -> per-core bf16 z arrays, channels-on-partition layout,
    x|y interleaved per chunk."""
    fc = F // nch

    def to_parts(A):
        A = np.asarray(A, dtype=np.float32).reshape(N_CORES, F, 4, C)
        # partition p = 32*(R%4) + c ; free col q = R//4
        return A.transpose(0, 2, 3, 1).reshape(N_CORES, P, F)

    Xp, Yp = to_parts(X), to_parts(Y)
    Z = np.empty((N_CORES, P, 2 * F), dtype=np.float32)
    for ch in range(nch):
        Z[:, :, 2 * ch * fc:(2 * ch + 1) * fc] = Xp[:, :, ch * fc:(ch + 1) * fc]
        Z[:, :, (2 * ch + 1) * fc:(2 * ch + 2) * fc] = Yp[:, :, ch * fc:(ch + 1) * fc]
    return Z.astype(ml_dtypes.float8_e4m3)


def kernel(X, Y, window=None, **_):
    global _nc_cache, _last_results
    Z = pack_inputs(X, Y)
    W = make_weights()
    if "k" not in _nc_cache:
        _nc_cache["k"] = _build()
    in_maps = [{"z": Z[k], "w": W} for k in range(N_CORES)]
    res = run_bass_kernel_spmd(_nc_cache["k"], in_maps, list(range(N_CORES)))
    _last_results = res
    partials = np.stack([r["out"] for r in res.results])  # (8, 128, 1)
    total = partials.astype(np.float64).sum()
    return np.float32(total / (B * N))
